# revision 46
# baseline (speedup 1.0000x reference)
"""Trainium2 Bass kernel for the SNN (LIF) network:

    cur1 = x.reshape(B,-1) @ W1.T + b1          (big fp32 matmul, once)
    200 sequential LIF steps on [B,1000] (layer 1), tiny matmul into 5
    outputs per step (layer 2), second LIF on [B,5].

Distribution over 8 cores:
  Phase A: contraction(K)-sharded exact-fp32 matmul (3 f16 passes:
           xh*wh + xh*wl + xl*wh) -> per-core partial cur1 [256, 1024],
           ReduceScatter(add) -> each core owns a 32-row batch slice.
  Phase B: per-core LIF layer-1 scan over its 32-batch slice, hidden on
           partitions ([128, 8chunks x 32batch] tiles). One custom DVE
           instruction per step writes into a 4-slot group buffer; one
           ACT Sign per 4 steps emits spikes g=sign(mem-1) in f16.
  Phase C: per group of 4 steps, PE contracts W2 chunks (stationary
           [128,5] f16) against g (moving [128, 4*32]) into PSUM [5,128];
           ACT copies out with scale=0.5 and bias b2+0.5*sum(W2) (folds
           spk=(1+g)/2).
  Phase D: layer-2 LIF scan on [5, 32] per step; spk2 = (mem2 > 1) at the
           end. Outputs [5, T*32] gathered on host.
"""
import sys

if "/opt/trn_rl_repo" not in sys.path:
    sys.path.insert(0, "/opt/trn_rl_repo")

import numpy as np
import ml_dtypes

# ---------------------------------------------------------------- constants
BETA = 0.95
T = 200
B = 256
NIN = 32000
NH = 1000
NO = 5

N_CORES = 8
KPAD = 32768           # NIN padded to 256*128
KC = KPAD // N_CORES   # 4096 contraction per core
KTILES = KC // 128     # 32
KB = 4                 # ktiles per weight DMA batch
NKB = KTILES // KB     # 8 weight DMA rounds
HPAD = 1024            # hidden padded
BLOC = B // N_CORES    # 32 batch rows per core after ReduceScatter
NCHUNK = HPAD // 128   # 8 hidden chunks of 128
G = 4                  # steps per sign/matmul group
NGROUP = T // G        # 50
W1SCALE = 256.0        # W1 pre-scale so the fp16 lo-half stays normal
CB = NCHUNK * BLOC     # 256 elements per partition per step

# ---------------------------------------------------------------- custom op
_LIF_NAME = "LIF_STEP_ANT"


def _register_lif_op():
    from concourse.dve_ops import (
        DveOp, OPS, CUSTOM_DVE_SPECS, _SUB_OPCODE_FOR_NAME, _CUSTOM_DVE_ROW_BASE,
    )
    from concourse.dve_spec import Spec, Src0, Src1, C0, One, lower as dve_lower, _has_src1
    from concourse.dve_uop import DveOpSpec

    for op in OPS:
        if op.name == _LIF_NAME:
            return op
    spec = Spec(
        body=Src0 * C0 + Src1 - (Src0 > One),
        reference=lambda in0, in1, s0: in0 * s0 + in1 - (in0 > 1.0).astype(np.float32),
    )
    if _LIF_NAME not in _SUB_OPCODE_FOR_NAME:
        _SUB_OPCODE_FOR_NAME[_LIF_NAME] = _CUSTOM_DVE_ROW_BASE + len(OPS)
    shas = {}
    for ver in ("v3", "v4"):
        s = DveOpSpec(
            name=_LIF_NAME,
            opcode=_SUB_OPCODE_FOR_NAME[_LIF_NAME],
            uops=dve_lower(spec, ver=ver),
            rd1_en=_has_src1(spec),
        )
        shas[ver] = s.sha(ver)
    op = DveOp(_LIF_NAME, spec, subdim=False, uops_sha=shas)
    OPS.append(op)
    CUSTOM_DVE_SPECS[_LIF_NAME] = op.spec
    return op


# ---------------------------------------------------------------- program
_PROGRAMS = {}  # sim -> (nc, lif_op)


def _build_program(sim=False):
    if sim in _PROGRAMS:
        return _PROGRAMS[sim]

    import concourse.bass as bass
    import concourse.tile as tile
    from concourse import bacc, mybir
    from concourse.masks import make_identity

    LIF = _register_lif_op()
    f32 = mybir.dt.float32
    f16 = mybir.dt.float16

    nc = bacc.Bacc("TRN2", target_bir_lowering=False, debug=False,
                   num_devices=1 if sim else N_CORES)

    # inputs (per-core); x/w1 pre-transposed on host so per-partition DMA
    # lines are long and contiguous.
    xth_d = nc.dram_tensor("xth", [128, KTILES * B], f16, kind="ExternalInput").ap()
    xtl_d = nc.dram_tensor("xtl", [128, KTILES * B], f16, kind="ExternalInput").ap()
    # half-major weight layout: [hidden-half][128][ktile][512] so each DMA's
    # per-partition run is contiguous (sz*1KB descriptors)
    w1h_d = nc.dram_tensor("w1h", [2, 128, KTILES, 512], f16, kind="ExternalInput").ap()
    w1l_d = nc.dram_tensor("w1l", [2, 128, KTILES, 512], f16, kind="ExternalInput").ap()
    b1c_d = nc.dram_tensor("b1c", [128, NCHUNK], f32, kind="ExternalInput").ap()
    w2h_d = nc.dram_tensor("w2h", [128, NCHUNK, NO], f16, kind="ExternalInput").ap()
    w2l_d = nc.dram_tensor("w2l", [128, NCHUNK, NO], f16, kind="ExternalInput").ap()
    b2e_d = nc.dram_tensor("b2e", [1, 2, NO], f16, kind="ExternalInput").ap()
    # outputs (per-core batch slice), free layout = (t, o)
    mem2_d = nc.dram_tensor("mem2rec", [BLOC, T * NO], f32, kind="ExternalOutput").ap()
    spk2_d = nc.dram_tensor("spk2rec", [BLOC, T * NO], f32, kind="ExternalOutput").ap()

    with tile.TileContext(nc) as tc:
        with (
            tc.tile_pool(name="win", bufs=3) as wpool,
            tc.tile_pool(name="psA", bufs=1, space="PSUM") as psA,
            tc.tile_pool(name="stage", bufs=1) as stage,
            tc.tile_pool(name="dram", bufs=1, space="DRAM") as dram,
            tc.tile_pool(name="g4", bufs=3) as gpool,
            tc.tile_pool(name="psC", bufs=2, space="PSUM") as psC,
            tc.tile_pool(name="psT", bufs=2, space="PSUM") as psT,
        ):
            # ------------- small constants + scan inputs issued up front so
            # DVE/ACT/SP do them during phase A
            xth_t = stage.tile([128, KTILES * B], f16, tag="xth")
            xtl_t = stage.tile([128, KTILES * B], f16, tag="xtl")
            XQ = 4  # x load chunks (first issued before weights, rest interleaved)
            xq = KTILES * B // XQ
            nc.sync.dma_start(xth_t[:, 0:xq], xth_d[:, 0:xq])
            nc.sync.dma_start(xtl_t[:, 0:xq], xtl_d[:, 0:xq])
            b1t = stage.tile([128, NCHUNK], f32, tag="b1t")
            nc.sync.dma_start(b1t[:], b1c_d[:])
            w2h_t = stage.tile([128, NCHUNK, NO], f16, tag="w2h")
            nc.sync.dma_start(w2h_t[:], w2h_d[:])
            w2l_t = stage.tile([128, NCHUNK, NO], f16, tag="w2l")
            nc.sync.dma_start(w2l_t[:], w2l_d[:])
            ones_t = stage.tile([1, 128], f16, tag="ones")
            nc.vector.memset(ones_t[:], 1.0)
            b2e_t = stage.tile([1, 2, NO], f16, tag="b2e")  # hi/lo f16 split
            nc.sync.dma_start(b2e_t[:], b2e_d[:])
            ident = stage.tile([BLOC, BLOC], f32, tag="ident")
            make_identity(nc, ident[:])
            biasm1 = stage.tile([128, 1], f32, tag="bm1")
            nc.vector.memset(biasm1[:], -1.0)
            zeros_t = stage.tile([128, CB], f32, tag="zeros")
            nc.vector.memset(zeros_t[:], 0.0)
            z32 = stage.tile([BLOC, NO], f32, tag="z32")
            nc.vector.memset(z32[:], 0.0)
            # tiny warm-up collective: absorbs the global barrier + first
            # cc-trigger latency so the real collectives start fast
            dumin = dram.tile([1, 32], f32, name="dumin")
            dumout = dram.tile([1, 32], f32, name="dumout")
            dz = stage.tile([1, 32], f32, tag="dz")
            nc.vector.memset(dz[:], 0.0)
            nc.sync.dma_start(dumin[:], dz[:])
            if not sim:
                nc.gpsimd.collective_compute(
                    "AllReduce",
                    mybir.AluOpType.add,
                    replica_groups=[list(range(N_CORES))],
                    ins=[dumin.opt()],
                    outs=[dumout.opt()],
                )

            # ---------------- phase A: cur1 partial = xT_slice.T @ W1T_slice
            # split into 2 hidden halves; each half's ReduceScatter overlaps
            # the next half's matmul.
            ps = [[psA.tile([128, 512], f32, tag=f"ps{mb}{nb}", name=f"ps{mb}{nb}")
                   for nb in range(2)] for mb in range(2)]
            xth_v = xth_t[:].rearrange("p (k b) -> p k b", b=B)
            xtl_v = xtl_t[:].rearrange("p (k b) -> p k b", b=B)
            # graduated weight batches: small first so the PE starts early
            batches = [(0, 1), (1, 1), (2, 2), (4, 4), (8, 4), (12, 4),
                       (16, 4), (20, 4), (24, 4), (28, 4)]
            # transpose targets prepared up front
            rsb = stage.tile([BLOC, HPAD], f32, tag="rsb")
            curb = stage.tile([128, CB], f32, tag="curb")

            def finalize_half(nbh):
                # per 256-col quarter: psum->SBUF copies, DMA to DRAM,
                # ReduceScatter, fetch + transpose into scan layout
                for q2 in range(2):
                    q = nbh * 2 + q2
                    cq = q2 * 256
                    partial = dram.tile([B, 256], f32, name=f"partial{q}")
                    for mb in range(2):
                        cs = stage.tile([128, 256], f32, tag=f"curp{mb}{q % 2}",
                                        name=f"cs{mb}{q}")
                        nc.scalar.activation(cs[:], ps[mb][nbh][:, cq:cq + 256],
                                             mybir.ActivationFunctionType.Copy,
                                             scale=1.0 / W1SCALE)
                        nc.sync.dma_start(partial[mb * 128:(mb + 1) * 128, :], cs[:])
                    rs_out = dram.tile([BLOC, 256], f32, name=f"rsout{q}")
                    if sim:
                        # timing stand-in for the collective (single-core sim)
                        nc.sync.dma_start(rs_out[:], partial[0:BLOC, :])
                    else:
                        nc.gpsimd.collective_compute(
                            "ReduceScatter",
                            mybir.AluOpType.add,
                            replica_groups=[list(range(N_CORES))],
                            ins=[partial.opt()],
                            outs=[rs_out.opt()],
                        )
                    nc.sync.dma_start(rsb[:, q * 256:(q + 1) * 256], rs_out[:])
                    for c in (2 * q, 2 * q + 1):
                        pt = psT.tile([128, BLOC], f32, tag="pst")
                        nc.tensor.transpose(pt[:], rsb[:, c * 128:(c + 1) * 128],
                                            ident[:])
                        nc.scalar.activation(
                            curb[:, c * BLOC:(c + 1) * BLOC], pt[:],
                            mybir.ActivationFunctionType.Identity,
                            bias=b1t[:, c:c + 1], scale=1.0,
                        )

            for nbh in range(2):
                c0 = nbh * 512
                for bi, (k0, sz) in enumerate(batches):
                    # alternate DMA rings (SP / ACT) to widen DMA bandwidth
                    eng_h = nc.sync if bi % 2 == 0 else nc.scalar
                    eng_l = nc.scalar if bi % 2 == 0 else nc.sync
                    wh_t = wpool.tile([128, sz, 512], f16, tag=f"w1h_s{sz}",
                                      name=f"wh{nbh}_{bi}")
                    eng_h.dma_start(wh_t[:], w1h_d[nbh, :, k0:k0 + sz, :])
                    wl_t = wpool.tile([128, sz, 512], f16, tag=f"w1l_s{sz}",
                                      name=f"wl{nbh}_{bi}")
                    eng_l.dma_start(wl_t[:], w1l_d[nbh, :, k0:k0 + sz, :])
                    if nbh == 0 and 1 <= bi <= 3:  # remaining x behind weights
                        q = bi
                        nc.sync.dma_start(xth_t[:, q * xq:(q + 1) * xq],
                                          xth_d[:, q * xq:(q + 1) * xq])
                        nc.sync.dma_start(xtl_t[:, q * xq:(q + 1) * xq],
                                          xtl_d[:, q * xq:(q + 1) * xq])
                    if nbh == 1 and bi == 6:
                        # half-0 finalize deferred here: its ReduceScatter then
                        # runs after most weight DMAs are issued (quieter HBM)
                        finalize_half(0)
                    for k2 in range(sz):
                        kt = k0 + k2
                        last = kt == KTILES - 1
                        for mb in range(2):
                            xh_s = xth_v[:, kt, mb * 128:(mb + 1) * 128]
                            xl_s = xtl_v[:, kt, mb * 128:(mb + 1) * 128]
                            out = ps[mb][nbh][:]
                            nc.tensor.matmul(out, xh_s, wl_t[:, k2, :],
                                             start=(kt == 0), stop=False)
                            nc.tensor.matmul(out, xh_s, wh_t[:, k2, :],
                                             start=False, stop=False)
                            nc.tensor.matmul(out, xl_s, wh_t[:, k2, :],
                                             start=False, stop=last)
            finalize_half(1)

            # ---------------- phase B/C: layer-1 scan + layer-2 matmul
            # mem slots: 2 alternating group buffers of 4 step-slots each
            NMB = 3
            mbuf = [stage.tile([128, G * CB], f32, tag=f"mbuf{g}", name=f"mbuf{g}")
                    for g in range(NMB)]
            cur2buf = stage.tile([128, NGROUP * NO], f32, tag="cur2buf")
            # cur2 rearranged to batch partitions: [b, t*5+o], filled in windows
            cur2r = stage.tile([BLOC, T * NO], f32, tag="cur2r")
            cur2r_v = cur2r[:].rearrange("p (g s o) -> p g s o", s=G, o=NO)
            GW = 4                 # groups per rearrange window
            LAG = GW * G + 12      # phase-D step t-LAG interleaves after scan step t
            mem2 = stage.tile([BLOC, T * NO], f32, tag="mem2")

            def rearrange_window(g0, g1):
                # cur2buf[sl*32+b, g*5+o] -> cur2r[b, ((g*4+sl)*5)+o] for g in [g0,g1)
                for sl in range(G):
                    nc.sync.dma_start(
                        cur2r_v[:, g0:g1, sl, :],
                        cur2buf[sl * BLOC:(sl + 1) * BLOC, g0 * NO:g1 * NO]
                        .rearrange("p (g o) -> p g o", o=NO),
                    )

            def phase_d_step(td):
                in0 = z32[:] if td == 0 else mem2[:, (td - 1) * NO:td * NO]
                nc.vector._custom_dve(
                    LIF,
                    out=mem2[:, td * NO:(td + 1) * NO],
                    in0=in0,
                    in1=cur2r[:, td * NO:(td + 1) * NO],
                    s0=BETA,
                )

            pcs = {}

            def cur2_copy(g):
                # psum -> cur2buf, deferred one group so it doesn't block the
                # next Sign on ACT behind the PE matmuls
                nc.scalar.activation(
                    cur2buf[:, g * NO:(g + 1) * NO], pcs.pop(g)[:],
                    mybir.ActivationFunctionType.Copy,
                )

            for t in range(1, T + 1):
                gi, sl = (t - 1) // G, (t - 1) % G
                gen = mbuf[gi % NMB]
                if t == 1:
                    prev = zeros_t[:]
                elif sl == 0:
                    prev = mbuf[(gi - 1) % NMB][:, (G - 1) * CB:]
                else:
                    prev = gen[:, (sl - 1) * CB:sl * CB]
                nc.vector._custom_dve(LIF, out=gen[:, sl * CB:(sl + 1) * CB],
                                      in0=prev, in1=curb[:], s0=BETA)
                if t > LAG:
                    phase_d_step(t - 1 - LAG)
                if sl == G - 1:
                    # one Sign over the whole group: stream order (sl, c, b)
                    gt = gpool.tile([128, NCHUNK, G, BLOC], f16, tag="gt")
                    nc.scalar.activation(
                        gt[:].rearrange("p c s b -> p s c b"),
                        gen[:].rearrange("p (s c b) -> p s c b", c=NCHUNK, b=BLOC),
                        mybir.ActivationFunctionType.Sign, bias=biasm1[:], scale=1.0,
                    )
                    # spikes stationary, W2 chunks moving (tiny N=5 streams)
                    pc = psC.tile([128, NO], f32, tag="psc")
                    pcs[gi] = pc
                    for c in range(NCHUNK):
                        lhs = gt[:, c, :, :].rearrange("p s b -> p (s b)")
                        nc.tensor.matmul(pc[:], lhs, w2h_t[:, c, :],
                                         start=(c == 0), stop=False)
                        nc.tensor.matmul(pc[:], lhs, w2l_t[:, c, :],
                                         start=False, stop=False)
                    nc.tensor.matmul(pc[:], ones_t[:], b2e_t[:, 0, :],
                                     start=False, stop=False)
                    nc.tensor.matmul(pc[:], ones_t[:], b2e_t[:, 1, :],
                                     start=False, stop=True)
                    if gi >= 1:
                        cur2_copy(gi - 1)
                    if gi >= GW and gi % GW == 0:
                        rearrange_window(gi - GW, gi)

            # ---------------- phase D tail: remaining interleaved steps
            cur2_copy(NGROUP - 1)
            rearrange_window(NGROUP - NGROUP % GW if NGROUP % GW else NGROUP - GW,
                             NGROUP)
            for td in range(T - LAG, T):
                phase_d_step(td)
            spk2 = stage.tile([BLOC, T * NO], f32, tag="spk2")
            nc.vector.tensor_scalar(spk2[:], mem2[:], 1.0, None, mybir.AluOpType.is_gt)
            nc.sync.dma_start(mem2_d[:], mem2[:])
            nc.sync.dma_start(spk2_d[:], spk2[:])

    nc.compile()
    _PROGRAMS[sim] = (nc, LIF)
    return _PROGRAMS[sim]


# ---------------------------------------------------------------- host prep
def _prep_inputs(x, W1, b1, W2, b2):
    f32 = np.float32
    x_flat = np.ascontiguousarray(x.reshape(B, -1).astype(f32, copy=False))  # [256, 32000]
    xT = np.zeros((KPAD, B), f32)
    xT[:NIN] = x_flat.T
    xTh = xT.astype(np.float16)
    xTl = (xT - xTh.astype(f32)).astype(np.float16)
    w1T = np.zeros((KPAD, HPAD), f32)
    w1T[:NIN, :NH] = W1.astype(f32, copy=False).T * W1SCALE
    w1Th = w1T.astype(np.float16)
    w1Tl = (w1T - w1Th.astype(f32)).astype(np.float16)
    b1p = np.full(HPAD, -10.0, f32)
    b1p[:NH] = b1
    b1c = np.ascontiguousarray(b1p.reshape(NCHUNK, 128).T)          # [128, 8]
    W2e = np.zeros((HPAD, NO), f32)
    W2e[:NH] = 0.5 * W2.astype(f32, copy=False).T
    w2stack = np.ascontiguousarray(W2e.reshape(NCHUNK, 128, NO).transpose(1, 0, 2))
    w2h = w2stack.astype(np.float16)                                 # [128,8,5]
    w2l = (w2stack - w2h.astype(f32)).astype(np.float16)
    b2f = (b2.astype(f32) + 0.5 * W2.astype(f32).sum(axis=1)).astype(f32)
    b2h = b2f.astype(np.float16)
    b2l = (b2f - b2h.astype(f32)).astype(np.float16)
    b2e = np.stack([b2h, b2l]).reshape(1, 2, NO)

    in_maps = []
    for c in range(N_CORES):
        ksl = slice(c * KC, (c + 1) * KC)
        # [4096, X] -> [32, 128, X] -> [128, 32, X] so per-partition DMA rows
        # are contiguous
        xh = np.ascontiguousarray(
            xTh[ksl].reshape(KTILES, 128, B).transpose(1, 0, 2)).reshape(128, KTILES * B)
        xl = np.ascontiguousarray(
            xTl[ksl].reshape(KTILES, 128, B).transpose(1, 0, 2)).reshape(128, KTILES * B)
        # [4096, 1024] -> [half, 128, ktile, 512] contiguous
        wh = np.ascontiguousarray(
            w1Th[ksl].reshape(KTILES, 128, 2, 512).transpose(2, 1, 0, 3))
        wl = np.ascontiguousarray(
            w1Tl[ksl].reshape(KTILES, 128, 2, 512).transpose(2, 1, 0, 3))
        in_maps.append({
            "xth": xh,
            "xtl": xl,
            "w1h": wh,
            "w1l": wl,
            "b1c": b1c,
            "w2h": w2h,
            "w2l": w2l,
            "b2e": b2e,
        })
    return in_maps


def _gather(results):
    spk_parts, mem_parts = [], []
    for r in results:
        mem_parts.append(r["mem2rec"].reshape(BLOC, T, NO).transpose(1, 0, 2))
        spk_parts.append(r["spk2rec"].reshape(BLOC, T, NO).transpose(1, 0, 2))
    mem2 = np.concatenate(mem_parts, axis=1).astype(np.float32)  # [200, 256, 5]
    spk2 = np.concatenate(spk_parts, axis=1).astype(np.float32)
    return spk2, mem2


def run_raw(inputs, **kwargs):
    """Build+run; returns BassKernelResults (for profiling from test.py)."""
    from concourse.bass_utils import run_bass_kernel_spmd

    nc, _ = _build_program()
    in_maps = _prep_inputs(**inputs)
    return run_bass_kernel_spmd(nc, in_maps, core_ids=list(range(N_CORES)), **kwargs)


def kernel(x, W1, b1, W2, b2):
    res = run_raw(dict(x=x, W1=W1, b1=b1, W2=W2, b2=b2))
    return _gather(res.results)


if __name__ == "__main__":
    rng = np.random.default_rng(0)
    ins = {
        "x": rng.standard_normal((B, 2, 80, 200)).astype(np.float32),
        "W1": rng.uniform(-1, 1, (NH, NIN)).astype(np.float32) / np.sqrt(NIN),
        "b1": rng.uniform(-1, 1, NH).astype(np.float32) / np.sqrt(NIN),
        "W2": rng.uniform(-1, 1, (NO, NH)).astype(np.float32) / np.sqrt(NH),
        "b2": rng.uniform(-1, 1, NO).astype(np.float32) / np.sqrt(NH),
    }
    spk2, mem2 = kernel(**ins)
    print("shapes:", spk2.shape, mem2.shape, spk2.dtype, mem2.dtype)
    print("spk2 mean:", spk2.mean(), "mem2 std:", mem2.std())


# revision 47
# speedup vs baseline: 1.0050x; 1.0050x over previous
"""Trainium2 Bass kernel for the SNN (LIF) network:

    cur1 = x.reshape(B,-1) @ W1.T + b1          (big fp32 matmul, once)
    200 sequential LIF steps on [B,1000] (layer 1), tiny matmul into 5
    outputs per step (layer 2), second LIF on [B,5].

Distribution over 8 cores:
  Phase A: contraction(K)-sharded exact-fp32 matmul (3 f16 passes:
           xh*wh + xh*wl + xl*wh) -> per-core partial cur1 [256, 1024],
           ReduceScatter(add) -> each core owns a 32-row batch slice.
  Phase B: per-core LIF layer-1 scan over its 32-batch slice, hidden on
           partitions ([128, 8chunks x 32batch] tiles). One custom DVE
           instruction per step writes into a 4-slot group buffer; one
           ACT Sign per 4 steps emits spikes g=sign(mem-1) in f16.
  Phase C: per group of 4 steps, PE contracts W2 chunks (stationary
           [128,5] f16) against g (moving [128, 4*32]) into PSUM [5,128];
           ACT copies out with scale=0.5 and bias b2+0.5*sum(W2) (folds
           spk=(1+g)/2).
  Phase D: layer-2 LIF scan on [5, 32] per step; spk2 = (mem2 > 1) at the
           end. Outputs [5, T*32] gathered on host.
"""
import sys

if "/opt/trn_rl_repo" not in sys.path:
    sys.path.insert(0, "/opt/trn_rl_repo")

import numpy as np
import ml_dtypes

# ---------------------------------------------------------------- constants
BETA = 0.95
T = 200
B = 256
NIN = 32000
NH = 1000
NO = 5

N_CORES = 8
KPAD = 32768           # NIN padded to 256*128
KC = KPAD // N_CORES   # 4096 contraction per core
KTILES = KC // 128     # 32
KB = 4                 # ktiles per weight DMA batch
NKB = KTILES // KB     # 8 weight DMA rounds
HPAD = 1024            # hidden padded
BLOC = B // N_CORES    # 32 batch rows per core after ReduceScatter
NCHUNK = HPAD // 128   # 8 hidden chunks of 128
G = 4                  # steps per sign/matmul group
NGROUP = T // G        # 50
W1SCALE = 256.0        # W1 pre-scale so the fp16 lo-half stays normal
CB = NCHUNK * BLOC     # 256 elements per partition per step

# ---------------------------------------------------------------- custom op
_LIF_NAME = "LIF_STEP_ANT"


def _register_lif_op():
    from concourse.dve_ops import (
        DveOp, OPS, CUSTOM_DVE_SPECS, _SUB_OPCODE_FOR_NAME, _CUSTOM_DVE_ROW_BASE,
    )
    from concourse.dve_spec import Spec, Src0, Src1, C0, One, lower as dve_lower, _has_src1
    from concourse.dve_uop import DveOpSpec

    for op in OPS:
        if op.name == _LIF_NAME:
            return op
    spec = Spec(
        body=Src0 * C0 + Src1 - (Src0 > One),
        reference=lambda in0, in1, s0: in0 * s0 + in1 - (in0 > 1.0).astype(np.float32),
    )
    if _LIF_NAME not in _SUB_OPCODE_FOR_NAME:
        _SUB_OPCODE_FOR_NAME[_LIF_NAME] = _CUSTOM_DVE_ROW_BASE + len(OPS)
    shas = {}
    for ver in ("v3", "v4"):
        s = DveOpSpec(
            name=_LIF_NAME,
            opcode=_SUB_OPCODE_FOR_NAME[_LIF_NAME],
            uops=dve_lower(spec, ver=ver),
            rd1_en=_has_src1(spec),
        )
        shas[ver] = s.sha(ver)
    op = DveOp(_LIF_NAME, spec, subdim=False, uops_sha=shas)
    OPS.append(op)
    CUSTOM_DVE_SPECS[_LIF_NAME] = op.spec
    return op


# ---------------------------------------------------------------- program
_PROGRAMS = {}  # sim -> (nc, lif_op)


def _build_program(sim=False):
    if sim in _PROGRAMS:
        return _PROGRAMS[sim]

    import concourse.bass as bass
    import concourse.tile as tile
    from concourse import bacc, mybir
    from concourse.masks import make_identity

    LIF = _register_lif_op()
    f32 = mybir.dt.float32
    f16 = mybir.dt.float16

    nc = bacc.Bacc("TRN2", target_bir_lowering=False, debug=False,
                   num_devices=1 if sim else N_CORES)

    # inputs (per-core); x/w1 pre-transposed on host so per-partition DMA
    # lines are long and contiguous.
    xth_d = nc.dram_tensor("xth", [128, KTILES * B], f16, kind="ExternalInput").ap()
    xtl_d = nc.dram_tensor("xtl", [128, KTILES * B], f16, kind="ExternalInput").ap()
    # half-major weight layout: [hidden-half][128][ktile][512] so each DMA's
    # per-partition run is contiguous (sz*1KB descriptors)
    w1h_d = nc.dram_tensor("w1h", [2, 128, KTILES, 512], f16, kind="ExternalInput").ap()
    w1l_d = nc.dram_tensor("w1l", [2, 128, KTILES, 512], f16, kind="ExternalInput").ap()
    b1c_d = nc.dram_tensor("b1c", [128, NCHUNK], f32, kind="ExternalInput").ap()
    w2h_d = nc.dram_tensor("w2h", [128, NCHUNK, NO], f16, kind="ExternalInput").ap()
    w2l_d = nc.dram_tensor("w2l", [128, NCHUNK, NO], f16, kind="ExternalInput").ap()
    b2e_d = nc.dram_tensor("b2e", [1, 2, NO], f16, kind="ExternalInput").ap()
    # outputs (per-core batch slice), free layout = (t, o)
    mem2_d = nc.dram_tensor("mem2rec", [BLOC, T * NO], f32, kind="ExternalOutput").ap()
    spk2_d = nc.dram_tensor("spk2rec", [BLOC, T * NO], f32, kind="ExternalOutput").ap()

    with tile.TileContext(nc) as tc:
        with (
            tc.tile_pool(name="win", bufs=3) as wpool,
            tc.tile_pool(name="psA", bufs=1, space="PSUM") as psA,
            tc.tile_pool(name="stage", bufs=1) as stage,
            tc.tile_pool(name="dram", bufs=1, space="DRAM") as dram,
            tc.tile_pool(name="g4", bufs=3) as gpool,
            tc.tile_pool(name="psC", bufs=2, space="PSUM") as psC,
            tc.tile_pool(name="psT", bufs=2, space="PSUM") as psT,
        ):
            # ------------- small constants + scan inputs issued up front so
            # DVE/ACT/SP do them during phase A
            xth_t = stage.tile([128, KTILES * B], f16, tag="xth")
            xtl_t = stage.tile([128, KTILES * B], f16, tag="xtl")
            XQ = 4  # x load chunks (first issued before weights, rest interleaved)
            xq = KTILES * B // XQ
            nc.sync.dma_start(xth_t[:, 0:xq], xth_d[:, 0:xq])
            nc.sync.dma_start(xtl_t[:, 0:xq], xtl_d[:, 0:xq])
            b1t = stage.tile([128, NCHUNK], f32, tag="b1t")
            nc.sync.dma_start(b1t[:], b1c_d[:])
            w2h_t = stage.tile([128, NCHUNK, NO], f16, tag="w2h")
            nc.sync.dma_start(w2h_t[:], w2h_d[:])
            w2l_t = stage.tile([128, NCHUNK, NO], f16, tag="w2l")
            nc.sync.dma_start(w2l_t[:], w2l_d[:])
            ones_t = stage.tile([1, 128], f16, tag="ones")
            nc.vector.memset(ones_t[:], 1.0)
            b2e_t = stage.tile([1, 2, NO], f16, tag="b2e")  # hi/lo f16 split
            nc.sync.dma_start(b2e_t[:], b2e_d[:])
            ident = stage.tile([BLOC, BLOC], f32, tag="ident")
            make_identity(nc, ident[:])
            biasm1 = stage.tile([128, 1], f32, tag="bm1")
            nc.vector.memset(biasm1[:], -1.0)
            zeros_t = stage.tile([128, CB], f32, tag="zeros")
            nc.vector.memset(zeros_t[:], 0.0)
            z32 = stage.tile([BLOC, NO], f32, tag="z32")
            nc.vector.memset(z32[:], 0.0)
            # tiny warm-up collective: absorbs the global barrier + first
            # cc-trigger latency so the real collectives start fast
            dumin = dram.tile([1, 32], f32, name="dumin")
            dumout = dram.tile([1, 32], f32, name="dumout")
            dz = stage.tile([1, 32], f32, tag="dz")
            nc.vector.memset(dz[:], 0.0)
            nc.sync.dma_start(dumin[:], dz[:])
            if not sim:
                nc.gpsimd.collective_compute(
                    "AllReduce",
                    mybir.AluOpType.add,
                    replica_groups=[list(range(N_CORES))],
                    ins=[dumin.opt()],
                    outs=[dumout.opt()],
                )

            # ---------------- phase A: cur1 partial = xT_slice.T @ W1T_slice
            # split into 2 hidden halves; each half's ReduceScatter overlaps
            # the next half's matmul.
            ps = [[psA.tile([128, 512], f32, tag=f"ps{mb}{nb}", name=f"ps{mb}{nb}")
                   for nb in range(2)] for mb in range(2)]
            xth_v = xth_t[:].rearrange("p (k b) -> p k b", b=B)
            xtl_v = xtl_t[:].rearrange("p (k b) -> p k b", b=B)
            # graduated weight batches: small first so the PE starts early
            batches = [(0, 1), (1, 1), (2, 2), (4, 4), (8, 4), (12, 4),
                       (16, 4), (20, 4), (24, 4), (28, 4)]
            # transpose targets prepared up front
            rsb = stage.tile([BLOC, HPAD], f32, tag="rsb")
            curb = stage.tile([128, CB], f32, tag="curb")

            def finalize_half(nbh):
                # psum->SBUF copies, DMA to DRAM, ReduceScatter, fetch +
                # transpose into scan layout
                partial = dram.tile([B, 512], f32, name=f"partial{nbh}")
                for mb in range(2):
                    cs = stage.tile([128, 512], f32, tag=f"curp{mb}",
                                    name=f"cs{mb}{nbh}")
                    nc.scalar.activation(cs[:], ps[mb][nbh][:],
                                         mybir.ActivationFunctionType.Copy,
                                         scale=1.0 / W1SCALE)
                    nc.sync.dma_start(partial[mb * 128:(mb + 1) * 128, :], cs[:])
                rs_out = dram.tile([BLOC, 512], f32, name=f"rsout{nbh}")
                if sim:
                    # timing stand-in for the collective (single-core sim)
                    nc.sync.dma_start(rs_out[:], partial[0:BLOC, :])
                else:
                    nc.gpsimd.collective_compute(
                        "ReduceScatter",
                        mybir.AluOpType.add,
                        replica_groups=[list(range(N_CORES))],
                        ins=[partial.opt()],
                        outs=[rs_out.opt()],
                    )
                nc.sync.dma_start(rsb[:, nbh * 512:(nbh + 1) * 512], rs_out[:])
                for c in range(4 * nbh, 4 * nbh + 4):
                    pt = psT.tile([128, BLOC], f32, tag="pst")
                    nc.tensor.transpose(pt[:], rsb[:, c * 128:(c + 1) * 128],
                                        ident[:])
                    nc.scalar.activation(
                        curb[:, c * BLOC:(c + 1) * BLOC], pt[:],
                        mybir.ActivationFunctionType.Identity,
                        bias=b1t[:, c:c + 1], scale=1.0,
                    )

            for nbh in range(2):
                c0 = nbh * 512
                for bi, (k0, sz) in enumerate(batches):
                    # alternate DMA rings (SP / ACT) to widen DMA bandwidth
                    eng_h = nc.sync if bi % 2 == 0 else nc.scalar
                    eng_l = nc.scalar if bi % 2 == 0 else nc.sync
                    wh_t = wpool.tile([128, sz, 512], f16, tag=f"w1h_s{sz}",
                                      name=f"wh{nbh}_{bi}")
                    eng_h.dma_start(wh_t[:], w1h_d[nbh, :, k0:k0 + sz, :])
                    wl_t = wpool.tile([128, sz, 512], f16, tag=f"w1l_s{sz}",
                                      name=f"wl{nbh}_{bi}")
                    eng_l.dma_start(wl_t[:], w1l_d[nbh, :, k0:k0 + sz, :])
                    if nbh == 0 and 1 <= bi <= 3:  # remaining x behind weights
                        q = bi
                        nc.sync.dma_start(xth_t[:, q * xq:(q + 1) * xq],
                                          xth_d[:, q * xq:(q + 1) * xq])
                        nc.sync.dma_start(xtl_t[:, q * xq:(q + 1) * xq],
                                          xtl_d[:, q * xq:(q + 1) * xq])
                    if nbh == 1 and bi == 6:
                        # half-0 finalize deferred here: its ReduceScatter then
                        # runs after most weight DMAs are issued (quieter HBM)
                        finalize_half(0)
                    for k2 in range(sz):
                        kt = k0 + k2
                        last = kt == KTILES - 1
                        for mb in range(2):
                            xh_s = xth_v[:, kt, mb * 128:(mb + 1) * 128]
                            xl_s = xtl_v[:, kt, mb * 128:(mb + 1) * 128]
                            out = ps[mb][nbh][:]
                            nc.tensor.matmul(out, xh_s, wl_t[:, k2, :],
                                             start=(kt == 0), stop=False)
                            nc.tensor.matmul(out, xh_s, wh_t[:, k2, :],
                                             start=False, stop=False)
                            nc.tensor.matmul(out, xl_s, wh_t[:, k2, :],
                                             start=False, stop=last)
            finalize_half(1)

            # ---------------- phase B/C: layer-1 scan + layer-2 matmul
            # mem slots: 2 alternating group buffers of 4 step-slots each
            NMB = 3
            mbuf = [stage.tile([128, G * CB], f32, tag=f"mbuf{g}", name=f"mbuf{g}")
                    for g in range(NMB)]
            cur2buf = stage.tile([128, NGROUP * NO], f32, tag="cur2buf")
            # cur2 rearranged to batch partitions: [b, t*5+o], filled in windows
            cur2r = stage.tile([BLOC, T * NO], f32, tag="cur2r")
            cur2r_v = cur2r[:].rearrange("p (g s o) -> p g s o", s=G, o=NO)
            GW = 4                 # groups per rearrange window
            LAG = GW * G + 12      # phase-D step t-LAG interleaves after scan step t
            mem2 = stage.tile([BLOC, T * NO], f32, tag="mem2")

            def rearrange_window(g0, g1):
                # cur2buf[sl*32+b, g*5+o] -> cur2r[b, ((g*4+sl)*5)+o] for g in [g0,g1)
                for sl in range(G):
                    nc.sync.dma_start(
                        cur2r_v[:, g0:g1, sl, :],
                        cur2buf[sl * BLOC:(sl + 1) * BLOC, g0 * NO:g1 * NO]
                        .rearrange("p (g o) -> p g o", o=NO),
                    )

            def phase_d_step(td):
                in0 = z32[:] if td == 0 else mem2[:, (td - 1) * NO:td * NO]
                nc.vector._custom_dve(
                    LIF,
                    out=mem2[:, td * NO:(td + 1) * NO],
                    in0=in0,
                    in1=cur2r[:, td * NO:(td + 1) * NO],
                    s0=BETA,
                )

            pcs = {}

            def cur2_copy(g):
                # psum -> cur2buf, deferred one group so it doesn't block the
                # next Sign on ACT behind the PE matmuls
                nc.scalar.activation(
                    cur2buf[:, g * NO:(g + 1) * NO], pcs.pop(g)[:],
                    mybir.ActivationFunctionType.Copy,
                )

            for t in range(1, T + 1):
                gi, sl = (t - 1) // G, (t - 1) % G
                gen = mbuf[gi % NMB]
                if t == 1:
                    prev = zeros_t[:]
                elif sl == 0:
                    prev = mbuf[(gi - 1) % NMB][:, (G - 1) * CB:]
                else:
                    prev = gen[:, (sl - 1) * CB:sl * CB]
                nc.vector._custom_dve(LIF, out=gen[:, sl * CB:(sl + 1) * CB],
                                      in0=prev, in1=curb[:], s0=BETA)
                if t > LAG:
                    phase_d_step(t - 1 - LAG)
                if sl == G - 1:
                    # one Sign over the whole group: stream order (sl, c, b)
                    gt = gpool.tile([128, NCHUNK, G, BLOC], f16, tag="gt")
                    nc.scalar.activation(
                        gt[:].rearrange("p c s b -> p s c b"),
                        gen[:].rearrange("p (s c b) -> p s c b", c=NCHUNK, b=BLOC),
                        mybir.ActivationFunctionType.Sign, bias=biasm1[:], scale=1.0,
                    )
                    # spikes stationary, W2 chunks moving (tiny N=5 streams)
                    pc = psC.tile([128, NO], f32, tag="psc")
                    pcs[gi] = pc
                    for c in range(NCHUNK):
                        lhs = gt[:, c, :, :].rearrange("p s b -> p (s b)")
                        nc.tensor.matmul(pc[:], lhs, w2h_t[:, c, :],
                                         start=(c == 0), stop=False)
                        nc.tensor.matmul(pc[:], lhs, w2l_t[:, c, :],
                                         start=False, stop=False)
                    nc.tensor.matmul(pc[:], ones_t[:], b2e_t[:, 0, :],
                                     start=False, stop=False)
                    nc.tensor.matmul(pc[:], ones_t[:], b2e_t[:, 1, :],
                                     start=False, stop=True)
                    if gi >= 1:
                        cur2_copy(gi - 1)
                    if gi >= GW and gi % GW == 0:
                        rearrange_window(gi - GW, gi)

            # ---------------- phase D tail: remaining interleaved steps
            cur2_copy(NGROUP - 1)
            rearrange_window(NGROUP - NGROUP % GW if NGROUP % GW else NGROUP - GW,
                             NGROUP)
            for td in range(T - LAG, T):
                phase_d_step(td)
            spk2 = stage.tile([BLOC, T * NO], f32, tag="spk2")
            nc.vector.tensor_scalar(spk2[:], mem2[:], 1.0, None, mybir.AluOpType.is_gt)
            nc.sync.dma_start(mem2_d[:], mem2[:])
            nc.sync.dma_start(spk2_d[:], spk2[:])

    nc.compile()
    _PROGRAMS[sim] = (nc, LIF)
    return _PROGRAMS[sim]


# ---------------------------------------------------------------- host prep
def _prep_inputs(x, W1, b1, W2, b2):
    f32 = np.float32
    x_flat = np.ascontiguousarray(x.reshape(B, -1).astype(f32, copy=False))  # [256, 32000]
    xT = np.zeros((KPAD, B), f32)
    xT[:NIN] = x_flat.T
    xTh = xT.astype(np.float16)
    xTl = (xT - xTh.astype(f32)).astype(np.float16)
    w1T = np.zeros((KPAD, HPAD), f32)
    w1T[:NIN, :NH] = W1.astype(f32, copy=False).T * W1SCALE
    w1Th = w1T.astype(np.float16)
    w1Tl = (w1T - w1Th.astype(f32)).astype(np.float16)
    b1p = np.full(HPAD, -10.0, f32)
    b1p[:NH] = b1
    b1c = np.ascontiguousarray(b1p.reshape(NCHUNK, 128).T)          # [128, 8]
    W2e = np.zeros((HPAD, NO), f32)
    W2e[:NH] = 0.5 * W2.astype(f32, copy=False).T
    w2stack = np.ascontiguousarray(W2e.reshape(NCHUNK, 128, NO).transpose(1, 0, 2))
    w2h = w2stack.astype(np.float16)                                 # [128,8,5]
    w2l = (w2stack - w2h.astype(f32)).astype(np.float16)
    b2f = (b2.astype(f32) + 0.5 * W2.astype(f32).sum(axis=1)).astype(f32)
    b2h = b2f.astype(np.float16)
    b2l = (b2f - b2h.astype(f32)).astype(np.float16)
    b2e = np.stack([b2h, b2l]).reshape(1, 2, NO)

    in_maps = []
    for c in range(N_CORES):
        ksl = slice(c * KC, (c + 1) * KC)
        # [4096, X] -> [32, 128, X] -> [128, 32, X] so per-partition DMA rows
        # are contiguous
        xh = np.ascontiguousarray(
            xTh[ksl].reshape(KTILES, 128, B).transpose(1, 0, 2)).reshape(128, KTILES * B)
        xl = np.ascontiguousarray(
            xTl[ksl].reshape(KTILES, 128, B).transpose(1, 0, 2)).reshape(128, KTILES * B)
        # [4096, 1024] -> [half, 128, ktile, 512] contiguous
        wh = np.ascontiguousarray(
            w1Th[ksl].reshape(KTILES, 128, 2, 512).transpose(2, 1, 0, 3))
        wl = np.ascontiguousarray(
            w1Tl[ksl].reshape(KTILES, 128, 2, 512).transpose(2, 1, 0, 3))
        in_maps.append({
            "xth": xh,
            "xtl": xl,
            "w1h": wh,
            "w1l": wl,
            "b1c": b1c,
            "w2h": w2h,
            "w2l": w2l,
            "b2e": b2e,
        })
    return in_maps


def _gather(results):
    spk_parts, mem_parts = [], []
    for r in results:
        mem_parts.append(r["mem2rec"].reshape(BLOC, T, NO).transpose(1, 0, 2))
        spk_parts.append(r["spk2rec"].reshape(BLOC, T, NO).transpose(1, 0, 2))
    mem2 = np.concatenate(mem_parts, axis=1).astype(np.float32)  # [200, 256, 5]
    spk2 = np.concatenate(spk_parts, axis=1).astype(np.float32)
    return spk2, mem2


def run_raw(inputs, **kwargs):
    """Build+run; returns BassKernelResults (for profiling from test.py)."""
    from concourse.bass_utils import run_bass_kernel_spmd

    nc, _ = _build_program()
    in_maps = _prep_inputs(**inputs)
    return run_bass_kernel_spmd(nc, in_maps, core_ids=list(range(N_CORES)), **kwargs)


def kernel(x, W1, b1, W2, b2):
    res = run_raw(dict(x=x, W1=W1, b1=b1, W2=W2, b2=b2))
    return _gather(res.results)


if __name__ == "__main__":
    rng = np.random.default_rng(0)
    ins = {
        "x": rng.standard_normal((B, 2, 80, 200)).astype(np.float32),
        "W1": rng.uniform(-1, 1, (NH, NIN)).astype(np.float32) / np.sqrt(NIN),
        "b1": rng.uniform(-1, 1, NH).astype(np.float32) / np.sqrt(NIN),
        "W2": rng.uniform(-1, 1, (NO, NH)).astype(np.float32) / np.sqrt(NH),
        "b2": rng.uniform(-1, 1, NO).astype(np.float32) / np.sqrt(NH),
    }
    spk2, mem2 = kernel(**ins)
    print("shapes:", spk2.shape, mem2.shape, spk2.dtype, mem2.dtype)
    print("spk2 mean:", spk2.mean(), "mem2 std:", mem2.std())
